# revision 21
# baseline (speedup 1.0000x reference)
"""Self-contained Trainium2 Bass kernel for nn_EncoderDecoderTransformer_90941637525663.

Strategy: sequence-parallel over 8 NeuronCores (2 batch groups x 4 token
shards of 256 tokens). Activations live TRANSPOSED in SBUF (feature dim on
partitions, tokens on free dim); weights stream in natural [in, out] layout
as the stationary operand. All heavy matmuls run in bf16 (full PE rate +
fast weight load); stats/broadcast matmuls run f32r. Residual stream kept
fp32 in SBUF with a bf16 shadow copy for matmul use. Per-head RMS norm of
q/k makes the preceding layernorm scale cancel, so q/k projections skip LN
entirely and ln2 is never computed. One bf16 AllGather per layer exchanges
self-attention K/V shards within each batch group, overlapped with the SA
q projection and CA k/v projections. Causal masking: keep-mask zeroes V
chunks for fully-masked history, own diagonal 256x256 block handled by a
second score pass with a post-exp binary triangular mask.
"""
import sys
sys.path.insert(0, '/opt/trn_rl_repo')
import numpy as np
import ml_dtypes

BF16 = ml_dtypes.bfloat16

B, TQ, TK, D, H, KVH, L, F = 2, 1024, 512, 1024, 16, 4, 2, 4096
HD, KVD = 64, 256
EPS = 1e-6
NCORES, TP = 8, 4
T = TQ // TP           # 256 tokens per core
DCH = D // 128         # 8 feature chunks


def _rope_tables(Tlen, hd, theta=10000.0):
    freqs = 1.0 / theta ** (np.arange(0, hd, 2, dtype=np.float32) / hd)
    ang = np.outer(np.arange(Tlen, dtype=np.float32), freqs)
    return np.cos(ang).astype(np.float32), np.sin(ang).astype(np.float32)


def host_prepare(inputs):
    """Returns (host, per_core): folded shared arrays + per-core arrays."""
    inputs = {k: np.ascontiguousarray(np.asarray(v, dtype=np.float32))
              for k, v in inputs.items()}
    cos_f, sin_f = _rope_tables(TQ, HD)       # [TQ, 32]

    def wlay(w, blk):
        """[Din, N] -> [128, N//blk, Din//128, blk]: per-(partition, block)
        contiguous lines so weight DMAs use big descriptors."""
        Din, N = w.shape
        K, nb = Din // 128, N // blk
        return np.ascontiguousarray(
            w.reshape(K, 128, nb, blk).transpose(1, 2, 0, 3)).astype(BF16)

    host = {}
    for i in range(L):
        ln1 = (1.0 + inputs['ln1_s'][i])[:, None]
        ln3 = (1.0 + inputs['ln3_s'][i])[:, None]
        # q/k rms-normalize per head, so any per-token LN scale would cancel;
        # the (identity here) ln column scales still fold into the weights.
        host[f'sa_wq_{i}'] = wlay(ln1 * inputs['sa_wq'][i], 512)
        host[f'sa_wk_{i}'] = wlay(ln1 * inputs['sa_wk'][i], KVD)
        host[f'sa_wv_{i}'] = wlay(ln1 * inputs['sa_wv'][i], KVD)
        host[f'sa_wo_{i}'] = wlay(inputs['sa_wo'][i], 512)
        host[f'ca_wq_{i}'] = wlay(inputs['ca_wq'][i], 512)
        host[f'ca_wk_{i}'] = wlay(inputs['ca_wk'][i], KVD)
        host[f'ca_wv_{i}'] = wlay(inputs['ca_wv'][i], KVD)
        host[f'ca_wo_{i}'] = wlay(inputs['ca_wo'][i], 512)
        host[f'ffn_wg_{i}'] = wlay(ln3 * inputs['ffn_wg'][i], 512)
        host[f'ffn_wu_{i}'] = wlay(ln3 * inputs['ffn_wu'][i], 512)
        # down-proj: [m][p][kc*128+n] contiguous per output-chunk slice
        wd = inputs['ffn_wd'][i]
        host[f'ffn_wd_{i}'] = np.ascontiguousarray(
            wd.reshape(F // 128, 128, DCH, 128).transpose(2, 1, 0, 3)
            .reshape(DCH, 128, F)).astype(BF16)
        for which, dvec in [('q', inputs['sa_qn'][i]), ('k', inputs['sa_kn'][i])]:
            d1, d2 = 1.0 + dvec[:32], 1.0 + dvec[32:]
            C = np.concatenate([d1[:, None] * cos_f.T, d2[:, None] * cos_f.T], 0)
            S = np.concatenate([-d2[:, None] * sin_f.T, d1[:, None] * sin_f.T], 0)
            # duplicated for head-pair tiles: [128, TQ]
            host[f'rope{which}_c_{i}'] = np.concatenate([C, C], 0)
            host[f'rope{which}_s_{i}'] = np.concatenate([S, S], 0)
        sc = ((1.0 + inputs['ca_qn'][i]) * (1.0 + inputs['ca_kn'][i])).astype(np.float32)
        host[f'ca_kscale_{i}'] = np.tile(sc, KVH)[:, None].copy()   # [256, 1]
    host['final_scale'] = (1.0 + inputs['final_s'])[:, None].copy()  # [D, 1]
    s2 = np.zeros((2, 128), np.float32)
    s2[0, 0:64] = 1.0
    s2[1, 64:128] = 1.0
    host['sel2const'] = s2

    # binary keep-mask for the own 256x256 causal block: [128, 2, 256] bf16
    kl = np.arange(T)[:, None]
    ql = np.arange(T)[None, :]
    tri = (kl <= ql).astype(np.float32)
    host['trimask'] = np.ascontiguousarray(
        tri.reshape(2, 128, T).transpose(1, 0, 2)).astype(BF16)

    per_core = []
    for c in range(NCORES):
        grp, r = c // TP, c % TP
        tok = slice(r * T, (r + 1) * T)
        pc = {
            'xT': np.ascontiguousarray(inputs['x'][grp].T[:, tok]),
            'xTb': np.ascontiguousarray(inputs['x'][grp].T[:, tok]).astype(BF16),
            'encT': np.ascontiguousarray(inputs['encoder_out'][grp].T).astype(BF16),
        }
        # keep-mask for pass-1 kv chunks: chunk (b, cb) kept iff 2b+cb < 2r
        vm = np.zeros((128, TP, 2), np.float32)
        for b in range(TP):
            for cb in range(2):
                vm[:, b, cb] = 1.0 if (2 * b + cb) < 2 * r else 0.0
        pc['vmask'] = vm.astype(BF16)
        for i in range(L):
            # one [128, 4, T] table per layer: (qc, qs, kc, ks)
            pc[f'rope_{i}'] = np.ascontiguousarray(np.stack(
                [host[f'ropeq_c_{i}'][:, tok], host[f'ropeq_s_{i}'][:, tok],
                 host[f'ropek_c_{i}'][:, tok], host[f'ropek_s_{i}'][:, tok]],
                axis=1)).astype(BF16)
        per_core.append(pc)
    for i in range(L):
        for which in ('q', 'k'):
            del host[f'rope{which}_c_{i}'], host[f'rope{which}_s_{i}']
    return host, per_core


_PROG = None
REPS = 1
NO_CC = False


def _build_program(nlayers=L, reps=None):
    import concourse.bass as bass
    import concourse.tile as tile
    from concourse import bacc, mybir
    from concourse.alu_op_type import AluOpType
    from contextlib import ExitStack

    R = mybir.dt.float32r
    FP = mybir.dt.float32
    BF = mybir.dt.bfloat16
    PF32 = mybir.dt.float32
    Exp = mybir.ActivationFunctionType.Exp
    Sqrt = mybir.ActivationFunctionType.Sqrt
    Square = mybir.ActivationFunctionType.Square
    Relu = mybir.ActivationFunctionType.Relu
    Copy = mybir.ActivationFunctionType.Copy

    nc = bacc.Bacc("TRN2", target_bir_lowering=False, debug=False,
                   num_devices=NCORES)

    din = {}
    def dri(name, shape, dt):
        din[name] = nc.dram_tensor(name, list(shape), dt, kind="ExternalInput")

    dri('xT', (D, T), FP)
    dri('xTb', (D, T), BF)
    dri('encT', (D, TK), BF)
    dri('trimask', (128, 2, T), BF)
    dri('vmask', (128, TP, 2), BF)
    dri('final_scale', (D, 1), FP)
    dri('sel2const', (2, 128), R)
    for i in range(nlayers):
        dri(f'sa_wq_{i}', (128, 2, DCH, 512), BF)
        dri(f'sa_wk_{i}', (128, 1, DCH, KVD), BF)
        dri(f'sa_wv_{i}', (128, 1, DCH, KVD), BF)
        dri(f'sa_wo_{i}', (128, 2, DCH, 512), BF)
        dri(f'ca_wq_{i}', (128, 2, DCH, 512), BF)
        dri(f'ca_wk_{i}', (128, 1, DCH, KVD), BF)
        dri(f'ca_wv_{i}', (128, 1, DCH, KVD), BF)
        dri(f'ca_wo_{i}', (128, 2, DCH, 512), BF)
        dri(f'ffn_wg_{i}', (128, F // 512, DCH, 512), BF)
        dri(f'ffn_wu_{i}', (128, F // 512, DCH, 512), BF)
        dri(f'ffn_wd_{i}', (DCH, 128, F), BF)
        dri(f'rope_{i}', (128, 4, T), BF)
        dri(f'ca_kscale_{i}', (KVD, 1), FP)
    out_dram = nc.dram_tensor('outT', [D, T], FP, kind="ExternalOutput")
    AGR = 2 * T + 2
    ag_in = [nc.dram_tensor(f'ag_in_{i}', [AGR, T], BF) for i in range(nlayers)]
    ag_out = [nc.dram_tensor(f'ag_out_{i}', [AGR * TP, T], BF) for i in range(nlayers)]
    own_stats = [nc.dram_tensor(f'own_stats_{i}', [T], FP) for i in range(nlayers)]
    GROUPS = [[0, 1, 2, 3], [4, 5, 6, 7]]

    with nc.allow_low_precision(reason="bf16 pipeline"), \
            tile.TileContext(nc) as tc, ExitStack() as ctx:
        consts = ctx.enter_context(tc.tile_pool(name="consts", bufs=1))
        state = ctx.enter_context(tc.tile_pool(name="state", bufs=1))
        kvf = ctx.enter_context(tc.tile_pool(name="kvf", bufs=1))
        wbig = ctx.enter_context(tc.tile_pool(name="wbig", bufs=4))
        wdp = ctx.enter_context(tc.tile_pool(name="wdp", bufs=2))
        wkv = ctx.enter_context(tc.tile_pool(name="wkv", bufs=3))
        workA = ctx.enter_context(tc.tile_pool(name="workA", bufs=2))
        workB = ctx.enter_context(tc.tile_pool(name="workB", bufs=1))
        psb = ctx.enter_context(tc.tile_pool(name="psb", bufs=5))
        ffnp = ctx.enter_context(tc.tile_pool(name="ffnp", bufs=2))
        ownp = ctx.enter_context(tc.tile_pool(name="ownp", bufs=1))
        prodp = ctx.enter_context(tc.tile_pool(name="prodp", bufs=1))
        ropep = ctx.enter_context(tc.tile_pool(name="ropep", bufs=2))
        ps = ctx.enter_context(tc.tile_pool(name="ps", bufs=8, space="PSUM"))

        def pst(p_, f_, name):
            return ps.tile([p_, f_], PF32, tag="psA", name=name)

        MUL, ADD = AluOpType.mult, AluOpType.add

        # ---- constants ----
        ones_r = consts.tile([128, 128], R, tag="ones_r", name="ones_r")
        nc.vector.memset(ones_r[:].bitcast(FP), 1.0)
        ones_b = consts.tile([128, 1], BF, tag="ones_b", name="ones_b")
        nc.vector.memset(ones_b[:], 1.0)
        bd_b = consts.tile([128, 2], BF, tag="bd_b", name="bd_b")
        nc.vector.memset(bd_b[:], 0.0)
        nc.vector.memset(bd_b[0:64, 0:1], 1.0)
        nc.vector.memset(bd_b[64:128, 1:2], 1.0)
        sel2 = consts.tile([2, 128], R, tag="sel2", name="sel2")
        nc.sync.dma_start(out=sel2[:], in_=din['sel2const'].ap())
        eps_t = consts.tile([128, 1], FP, tag="eps", name="eps")
        nc.vector.memset(eps_t[:], EPS)
        trimask = consts.tile([128, 2 * T], BF, tag="trimask", name="trimask")
        nc.sync.dma_start(out=trimask[:],
                          in_=din['trimask'].ap().rearrange("p a b -> p (a b)"))
        vmask = consts.tile([128, TP, 2], BF, tag="vmask", name="vmask")
        nc.sync.dma_start(out=vmask[:], in_=din['vmask'].ap())
        fscale = consts.tile([128, DCH], FP, tag="fscale", name="fscale")
        nc.sync.dma_start(out=fscale[:],
                          in_=din['final_scale'].ap().rearrange("(k p) o -> p (k o)", p=128))

        # ---- persistent state ----
        x = [state.tile([128, T], FP, tag=f"x{m}", name=f"x{m}") for m in range(DCH)]
        xb = [state.tile([128, T], BF, tag=f"xb{m}", name=f"xb{m}") for m in range(DCH)]
        for m in range(DCH):
            nc.sync.dma_start(out=xb[m][:], in_=din['xTb'].ap()[128 * m:128 * (m + 1), :])
        enc = [state.tile([128, TK], BF, tag=f"enc{m}", name=f"enc{m}") for m in range(DCH)]
        first_load = [True]
        ao = [state.tile([128, T], BF, tag=f"ao{m}", name=f"ao{m}") for m in range(DCH)]
        qf = [state.tile([128, T], BF, tag=f"qf{t}", name=f"qf{t}") for t in range(H // 2)]
        kdup = [state.tile([128, T], BF, tag=f"kd{k}", name=f"kd{k}") for k in range(KVH)]
        v_own = [state.tile([128, 2, 65], BF, tag=f"vo{k}", name=f"vo{k}") for k in range(KVH)]
        kdca = [state.tile([128, TK], BF, tag=f"kdca{k}", name=f"kdca{k}") for k in range(KVH)]
        cv = [state.tile([128, TP, 65], BF, tag=f"cv{k}", name=f"cv{k}") for k in range(KVH)]

        # ---------------- helpers ----------------
        def ln_ss(src_tiles):
            """Sum of squares over D partitions -> [1,T] psum."""
            ss = pst(1, T, "ss")
            for m in range(DCH):
                sq = workA.tile([128, T], BF, tag="sq", name="sq")
                nc.scalar.activation(sq[:], src_tiles[m][:], Square)
                nc.tensor.matmul(ss[:], lhsT=ones_b[:, 0:1], rhs=sq[:],
                                 start=(m == 0), stop=(m == DCH - 1))
            return ss

        def ln_rinv(src_tiles, n):
            """1/rms over D partitions -> rinv [1,T] fp32."""
            ss = ln_ss(src_tiles)
            sr = workB.tile([1, T], FP, tag="sr", name="sr")
            nc.scalar.activation(sr[:], ss[:], Sqrt, bias=eps_t[0:1, :], scale=1.0 / n)
            rinv = workB.tile([1, T], FP, tag="rinv", name="rinv")
            nc.vector.reciprocal_approx_fast(out=rinv[:], in_=sr[:])
            return rinv

        def ln_rinv_bc(src_tiles, n):
            """1/rms over D partitions, broadcast -> [128,T] fp32 sbuf."""
            ss = ln_ss(src_tiles)
            srr = workB.tile([1, T], R, tag="srr", name="srr")
            nc.scalar.activation(srr[:], ss[:], Sqrt, bias=eps_t[0:1, :], scale=1.0 / n)
            bc_ps = pst(128, T, "bc")
            nc.tensor.matmul(bc_ps[:], lhsT=ones_r[0:1, :], rhs=srr[:],
                             start=True, stop=True)
            rbc = workA.tile([128, T], FP, tag="rbcf", name="rbcf")
            nc.vector.reciprocal_approx_fast(out=rbc[:], in_=bc_ps[:])
            return rbc

        def head_pair_rms(q_ps, width):
            """Per-head 1/rms of a [128, width] psum (2 heads) -> fp32 bcast
            [128, width] sbuf tile."""
            sq = workA.tile([128, width], BF, tag="sqh", name="sqh", padded_shape=[128, TK])
            nc.scalar.activation(sq[:], q_ps[:], Square)
            ssq = pst(2, width, "ssq")
            nc.tensor.matmul(ssq[:], lhsT=bd_b[:], rhs=sq[:], start=True, stop=True)
            sr = workB.tile([2, width], R, tag="srh", name="srh", padded_shape=[2, TK])
            nc.scalar.activation(sr[:], ssq[:], Sqrt, bias=eps_t[0:2, :], scale=1.0 / HD)
            bc_ps = pst(128, width, "bch")
            nc.tensor.matmul(bc_ps[:], lhsT=sel2[:], rhs=sr[:], start=True, stop=True)
            rbc = workA.tile([128, width], FP, tag="rbch", name="rbch", padded_shape=[128, TK])
            nc.vector.reciprocal_approx_fast(out=rbc[:], in_=bc_ps[:])
            return rbc

        def apply_rope(dst, qhat, c_t, s_t):
            qsw = workA.tile([128, T], BF, tag="qsw", name="qsw")
            for base in (0, 64):
                nc.gpsimd.dma_start(out=qsw[base:base + 32, :],
                                    in_=qhat[base + 32:base + 64, :])
                nc.gpsimd.dma_start(out=qsw[base + 32:base + 64, :],
                                    in_=qhat[base:base + 32, :])
            nc.vector.tensor_tensor(qhat[:], qhat[:], c_t, MUL)
            nc.vector.tensor_tensor(qsw[:], qsw[:], s_t, MUL)
            nc.vector.tensor_tensor(dst, qhat[:], qsw[:], ADD)

        def proj(dst_eval, w_name, ncols, blk=512):
            nblk = ncols // blk
            for bki in range(nblk):
                pool_, tg = (wbig, "wbig") if blk == 512 else (wkv, "wkv")
                wt = pool_.tile([128, DCH, blk], BF, tag=tg, name=tg)
                nc.sync.dma_start(out=wt[:], in_=din[w_name].ap()[:, bki])
                for j in range(blk // 128):
                    mt = bki * (blk // 128) + j
                    q_ps = pst(128, T, "proj")
                    for k in range(DCH):
                        nc.tensor.matmul(q_ps[:], lhsT=wt[:, k, 128 * j:128 * (j + 1)],
                                         rhs=xb[k][:],
                                         start=(k == 0), stop=(k == DCH - 1))
                    dst_eval(mt, q_ps)

        def own_precompute(i):
            """Exp'd+masked own-diagonal probability tiles for all 16 heads.
            Emitted before any AllGather-dependent matmul so the PE queue has
            work during the collective."""
            p2_own = {}
            for kv in range(KVH):
                for pr in range(2):
                    qt = kv * 2 + pr
                    s_pair = [pst(128, 512, "s"), pst(128, 512, "s")]
                    for cc in range(2):
                        for hp in range(2):
                            par = 64 * hp
                            nc.tensor.matmul(
                                s_pair[hp][:, 256 * cc:256 * (cc + 1)],
                                lhsT=kdup[kv][par:par + 64,
                                              128 * cc:128 * (cc + 1)],
                                rhs=qf[qt][par:par + 64, :],
                                start=True, stop=True)
                    for hp in range(2):
                        pe = psb.tile([128, 512], BF, tag="pe", name="pe", bufs=3)
                        nc.scalar.activation(pe[:], s_pair[hp][:], Exp, scale=0.125)
                        p2 = ownp.tile([128, 512], BF, tag=f"p2o{qt}_{hp}",
                                       name=f"p2o{qt}_{hp}")
                        nc.vector.tensor_tensor(p2[:], pe[:], trimask[:], MUL)
                        p2_own[2 * qt + hp] = p2
            attention.p2_own = p2_own

        def attention(i, is_sa):
            pending = []

            def flush_norm():
                for qt, hp, o_ps in pending:
                    par = 64 * hp
                    r_sb = workB.tile([65, T], R, tag="r_sb", name="r_sb", bufs=3)
                    nc.scalar.activation(r_sb[64:65, :], o_ps[64:65, :], Copy)
                    b_ps = pst(64, T, "b")
                    nc.tensor.matmul(b_ps[:], lhsT=ones_r[64:65, 0:64],
                                     rhs=r_sb[64:65, :], start=True, stop=True)
                    b_sb = workB.tile([64, T], FP, tag="b_sb", name="b_sb", bufs=3)
                    nc.vector.reciprocal_approx_fast(out=b_sb[:], in_=b_ps[:])
                    o_scr = workB.tile([64, T], BF, tag="o_scr", name="o_scr",
                                       bufs=2)
                    nc.vector.tensor_tensor(o_scr[:], o_ps[0:64, :], b_sb[:], MUL)
                    nc.gpsimd.dma_start(out=ao[qt][par:par + 64, :], in_=o_scr[:])
                pending.clear()

            for kv in range(KVH):
                if is_sa:
                    ag = ag_out[i].ap()
                    ksrc = kvf.tile([128, TQ], BF, tag="kfull", name="kfull")
                    src = bass.AP(tensor=ag.tensor, offset=(64 * kv) * T,
                                  ap=[[T, 64], [AGR * T, TP], [1, T]])
                    for dd in range(2):
                        nc.sync.dma_start(
                            out=ksrc[64 * dd:64 * (dd + 1), :].rearrange(
                                "p (b t) -> p b t", b=TP), in_=src)
                    vsrc = kvf.tile([128, TP, 2, 65], BF, tag="vfull", name="vfull")
                    for cb in range(2):
                        vap = bass.AP(tensor=ag.tensor,
                                      offset=T * T + 128 * T * cb + 64 * kv,
                                      ap=[[T, 128], [AGR * T, TP], [1, 64]])
                        nc.sync.dma_start(out=vsrc[:, :, cb, 0:64], in_=vap)
                    if kv == 0:
                        rvg = workB.tile([128, TP, 2], BF, tag="rvg", name="rvg")
                        for cb in range(2):
                            rap = bass.AP(tensor=ag.tensor, offset=2 * T * T + 128 * cb,
                                          ap=[[1, 128], [AGR * T, TP]])
                            nc.sync.dma_start(out=rvg[:, :, cb], in_=rap)
                        nc.vector.tensor_tensor(rvg[:], rvg[:], vmask[:], MUL)
                        attention.rvg = rvg
                    rvg = attention.rvg
                    nc.vector.tensor_tensor(
                        vsrc[:, :, :, 0:64], vsrc[:, :, :, 0:64],
                        rvg[:, :, :, None].broadcast_to([128, TP, 2, 64]), MUL)
                    nc.vector.tensor_tensor(
                        vsrc[:, :, :, 64:65], vmask[:, :, :, None],
                        vmask[:, :, :, None], MUL)
                    nk_chunks = TQ // 128
                else:
                    ksrc = kdca[kv]
                    vsrc = cv[kv]
                    nk_chunks = TK // 128
                # two heads (par 0 / par 64) pipelined: score matmuls of a
                # stage issue back-to-back on disjoint PE row groups (runs
                # concurrently), o-accumulation trails one stage behind so
                # the softmax exp is off the PE critical path. The own
                # diagonal block (precomputed) is the last accumulation stage.
                for pr in range(2):
                    qt = kv * 2 + pr
                    o_pair = [None, None]
                    nmm = nk_chunks + (2 if is_sa else 0)
                    stages = [('hist', h) for h in range(nk_chunks // 2)] + \
                        (['own'] if is_sa else [])
                    nst = len(stages)
                    mi = [0, 0]
                    prev = None
                    for si in range(nst + 1):
                        cur = None
                        if si < nst:
                            st = stages[si]
                            if st == 'own':
                                cur = (st, [attention.p2_own[2 * qt],
                                            attention.p2_own[2 * qt + 1]])
                            else:
                                s_pair = [pst(128, 512, "s"), pst(128, 512, "s")]
                                for cc in range(2):
                                    for hp in range(2):
                                        par = 64 * hp
                                        c = 2 * st[1] + cc
                                        nc.tensor.matmul(
                                            s_pair[hp][:, 256 * cc:256 * (cc + 1)],
                                            lhsT=ksrc[par:par + 64,
                                                      128 * c:128 * (c + 1)],
                                            rhs=qf[qt][par:par + 64, :],
                                            start=True, stop=True)
                                p_pair = []
                                for hp in range(2):
                                    p_sb = psb.tile([128, 512], BF, tag="p_sb",
                                                    name="p_sb")
                                    nc.scalar.activation(p_sb[:], s_pair[hp][:],
                                                         Exp, scale=0.125)
                                    p_pair.append(p_sb)
                                cur = (st, p_pair)
                        if si == 0:
                            flush_norm()   # previous pair's normalize queues
                                           # behind this pair's first scores
                        if prev is not None:
                            pst_, pp = prev
                            if o_pair[0] is None:
                                o_pair[0] = pst(65, T, "o")
                                o_pair[1] = pst(65, T, "o")
                            for hp in range(2):
                                for cc in range(2):
                                    if pst_ == 'own':
                                        vsl = v_own[kv][:, cc, :]
                                    else:
                                        c = 2 * pst_[1] + cc
                                        vsl = (vsrc[:, c // 2, c % 2, :]
                                               if is_sa else vsrc[:, c, :])
                                    nc.tensor.matmul(
                                        o_pair[hp][:],
                                        lhsT=vsl,
                                        rhs=pp[hp][:, 256 * cc:256 * (cc + 1)],
                                        start=(mi[hp] == 0),
                                        stop=(mi[hp] == nmm - 1),
                                        skip_group_check=True)
                                    mi[hp] += 1
                        prev = cur
                    for hp in range(2):
                        pending.append((qt, hp, o_pair[hp]))
            flush_norm()

        def stream_out_proj(w_name):
            for bki in range(2):
                wt = wbig.tile([128, DCH, 512], BF, tag="wbig", name="wbig")
                nc.sync.dma_start(out=wt[:], in_=din[w_name].ap()[:, bki])
                for j in range(4):
                    m = bki * 4 + j
                    y_ps = pst(128, T, "proj")
                    for k in range(DCH):
                        nc.tensor.matmul(y_ps[:], lhsT=wt[:, k, 128 * j:128 * (j + 1)],
                                         rhs=ao[k][:],
                                         start=(k == 0), stop=(k == DCH - 1))
                    nc.vector.tensor_tensor(x[m][:], x[m][:], y_ps[:], ADD)
                    nc.scalar.activation(xb[m][:], x[m][:], Copy)

        # ================= layers (REPS > 1 only for timing runs) =================
        for rep in range(reps if reps is not None else REPS):
          if rep > 0:
            for m in range(DCH):
                nc.sync.dma_start(out=x[m][:], in_=din['xT'].ap()[128 * m:128 * (m + 1), :])
                nc.sync.dma_start(out=xb[m][:], in_=din['xTb'].ap()[128 * m:128 * (m + 1), :])
          for i in range(nlayers):
              # ---- LN1 stats (only needed for v scaling) ----
              rinv = ln_rinv(xb, D)
              nc.sync.dma_start(out=own_stats[i].ap()[None, :], in_=rinv[:])
              rin_b = workB.tile([1, T], BF, tag="rin_b", name="rin_b")
              nc.vector.tensor_copy(out=rin_b[:], in_=rinv[:])
              nc.sync.dma_start(out=ag_in[i].ap()[2 * T:2 * T + 1, :], in_=rin_b[:])

              ropet = ropep.tile([128, 4, T], BF, tag="ropet", name="ropet")
              nc.gpsimd.dma_start(out=ropet[:], in_=din[f'rope_{i}'].ap())

              # ---- SA K projection (LN cancels under per-head rms) ----
              kf_pair = [workB.tile([128, T], BF, tag=f"kfp{t}", name=f"kfp{t}")
                         for t in range(2)]
              def k_eval(t, k_ps):
                  rbch = head_pair_rms(k_ps, T)
                  khat = workA.tile([128, T], BF, tag="qhat", name="qhat")
                  nc.vector.tensor_tensor(khat[:], k_ps[:], rbch[:], MUL)
                  apply_rope(kf_pair[t][:], khat, ropet[:, 2, :], ropet[:, 3, :])
              proj(k_eval, f'sa_wk_{i}', KVD, blk=KVD)
              for t in range(2):
                  nc.sync.dma_start(out=ag_in[i].ap()[128 * t:128 * (t + 1), :],
                                    in_=kf_pair[t][:])
                  for half in range(2):
                      kv = 2 * t + half
                      for dd in range(2):
                          nc.gpsimd.dma_start(out=kdup[kv][64 * dd:64 * (dd + 1), :],
                                              in_=kf_pair[t][64 * half:64 * (half + 1), :])

              # ---- SA V projection ----
              wvt = wkv.tile([128, DCH, KVD], BF, tag="wkv", name="wkv")
              nc.sync.dma_start(out=wvt[:], in_=din[f'sa_wv_{i}'].ap()[:, 0])
              for j in range(2):
                  v_ps = pst(128, KVD, "proj")
                  for k in range(DCH):
                      nc.tensor.matmul(v_ps[:], lhsT=xb[k][:, 128 * j:128 * (j + 1)],
                                       rhs=wvt[:, k, :], start=(k == 0), stop=(k == DCH - 1))
                  for kv in range(KVH):
                      nc.scalar.activation(v_own[kv][:, j, 0:64],
                                           v_ps[:, 64 * kv:64 * (kv + 1)], Copy)
              # own-block v scaling by rinv (token-on-partition, via dram bounce)
              for j in range(2):
                  rvT = workB.tile([128, 1], FP, tag="rvT", name="rvT", bufs=2)
                  nc.gpsimd.dma_start(out=rvT[:],
                                    in_=own_stats[i].ap()[128 * j:128 * (j + 1), None])
                  for kv in range(KVH):
                      nc.vector.tensor_scalar(
                          out=v_own[kv][:, j, 0:64],
                          in0=v_own[kv][:, j, 0:64],
                          scalar1=rvT[:], scalar2=None, op0=MUL)
              for kv in range(KVH):
                  nc.vector.memset(v_own[kv][:, :, 64:65], 1.0)
              for kv in range(KVH):
                  for j in range(2):
                      nc.gpsimd.dma_start(
                          out=ag_in[i].ap()[T + 128 * j:T + 128 * (j + 1),
                                            64 * kv:64 * (kv + 1)],
                          in_=v_own[kv][:, j, 0:64])

              if NO_CC:
                  for b in range(TP):
                      nc.sync.dma_start(
                          out=ag_out[i].ap()[AGR * b:AGR * (b + 1), :],
                          in_=ag_in[i].ap())
              else:
                  nc.gpsimd.collective_compute(
                      "AllGather", mybir.AluOpType.bypass, replica_groups=GROUPS,
                      ins=[ag_in[i].ap().opt()], outs=[ag_out[i].ap().opt()])
              if first_load[0]:
                  first_load[0] = False
                  for m in range(DCH):
                      nc.sync.dma_start(out=x[m][:],
                                        in_=din['xT'].ap()[128 * m:128 * (m + 1), :])
                      nc.sync.dma_start(out=enc[m][:],
                                        in_=din['encT'].ap()[128 * m:128 * (m + 1), :])

              # ---- overlap the AllGather: SA Q proj + CA K/V (enc-only) ----
              def q_eval(t, q_ps):
                  rbch = head_pair_rms(q_ps, T)
                  qhat = workA.tile([128, T], BF, tag="qhat", name="qhat")
                  nc.vector.tensor_tensor(qhat[:], q_ps[:], rbch[:], MUL)
                  apply_rope(qf[t][:], qhat, ropet[:, 0, :], ropet[:, 1, :])
              proj(q_eval, f'sa_wq_{i}', D)
              own_precompute(i)

              ksc = workB.tile([128, 2], FP, tag="ksc", name="ksc")
              nc.gpsimd.dma_start(out=ksc[:],
                                in_=din[f'ca_kscale_{i}'].ap().rearrange("(t p) o -> p (t o)", p=128))
              wkt = wkv.tile([128, DCH, KVD], BF, tag="wkv", name="wkv")
              nc.sync.dma_start(out=wkt[:], in_=din[f'ca_wk_{i}'].ap()[:, 0])
              for t in range(2):
                  k_ps = pst(128, TK, "s")
                  for k in range(DCH):
                      nc.tensor.matmul(k_ps[:], lhsT=wkt[:, k, 128 * t:128 * (t + 1)],
                                       rhs=enc[k][:], start=(k == 0), stop=(k == DCH - 1))
                  rbch = head_pair_rms(k_ps, TK)
                  kh = workB.tile([128, TK], BF, tag="khca", name="khca")
                  nc.vector.tensor_tensor(kh[:], k_ps[:], rbch[:], MUL)
                  ckp = workB.tile([128, TK], BF, tag=f"ckp{t}", name=f"ckp{t}")
                  nc.vector.tensor_scalar(
                      out=ckp[:], in0=kh[:],
                      scalar1=ksc[:, t:t + 1], scalar2=None, op0=MUL)
                  for half in range(2):
                      kv = 2 * t + half
                      for dd in range(2):
                          nc.gpsimd.dma_start(out=kdca[kv][64 * dd:64 * (dd + 1), :],
                                              in_=ckp[64 * half:64 * (half + 1), :])

              wvt2 = wkv.tile([128, DCH, KVD], BF, tag="wkv", name="wkv")
              nc.sync.dma_start(out=wvt2[:], in_=din[f'ca_wv_{i}'].ap()[:, 0])
              for kv in range(KVH):
                  nc.vector.memset(cv[kv][:, :, 64:65], 1.0)
              for j in range(TP):
                  v_ps = pst(128, KVD, "proj")
                  for k in range(DCH):
                      nc.tensor.matmul(v_ps[:], lhsT=enc[k][:, 128 * j:128 * (j + 1)],
                                       rhs=wvt2[:, k, :], start=(k == 0), stop=(k == DCH - 1))
                  for kv in range(KVH):
                      nc.scalar.activation(cv[kv][:, j, 0:64],
                                           v_ps[:, 64 * kv:64 * (kv + 1)], Copy)

              # ---- SA attention + out-proj ----
              attention(i, True)
              stream_out_proj(f'sa_wo_{i}')

              # ---- CA Q (LN2 cancels entirely) + attention + out-proj ----
              def cq_eval(t, q_ps):
                  rbch = head_pair_rms(q_ps, T)
                  nc.vector.tensor_tensor(qf[t][:], q_ps[:], rbch[:], MUL)
              proj(cq_eval, f'ca_wq_{i}', D)
              attention(i, False)
              stream_out_proj(f'ca_wo_{i}')

              # ---- LN3 + FFN ----
              rbc3 = ln_rinv_bc(xb, D)
              rv2_bc = workA.tile([128, T], BF, tag="rv2bc", name="rv2bc")
              nc.vector.tensor_tensor(rv2_bc[:], rbc3[:], rbc3[:], MUL)

              prods = []
              NF = F // 512
              for fb in range(NF):
                  wgt = wbig.tile([128, DCH, 512], BF, tag="wbig", name="wbig")
                  nc.sync.dma_start(out=wgt[:], in_=din[f'ffn_wg_{i}'].ap()[:, fb])
                  wut = wbig.tile([128, DCH, 512], BF, tag="wbig", name="wbig")
                  nc.sync.dma_start(out=wut[:], in_=din[f'ffn_wu_{i}'].ap()[:, fb])
                  for hf in range(2):
                      gu = []
                      for which, wt in (('g', wgt), ('u', wut)):
                          g_ps = pst(128, 512, "s")
                          for jj in range(2):
                              j = 2 * hf + jj
                              for k in range(DCH):
                                  nc.tensor.matmul(
                                      g_ps[:, 256 * jj:256 * (jj + 1)],
                                      lhsT=wt[:, k, 128 * j:128 * (j + 1)],
                                      rhs=xb[k][:], start=(k == 0), stop=(k == DCH - 1))
                          g_sb = ffnp.tile([128, 512], BF, tag=f"relu{which}", name=f"relu{which}")
                          nc.scalar.activation(g_sb[:], g_ps[:], Relu)
                          gu.append(g_sb)
                      pr = prodp.tile([128, 512], BF, tag=f"prod{fb}_{hf}",
                                      name=f"prod{fb}_{hf}")
                      nc.vector.tensor_tensor(pr[:], gu[0][:], gu[1][:], MUL)
                      prods.append(pr)
              # down-proj: m-outer, full-K accumulation (no partial adds)
              for m in range(DCH):
                  wdt = wdp.tile([128, F // 128, 128], BF, tag="wdp", name="wdp")
                  nc.sync.dma_start(
                      out=wdt[:].rearrange("p a b -> p (a b)"),
                      in_=din[f'ffn_wd_{i}'].ap()[m])
                  yp = pst(128, T, "yp")
                  for kc in range(F // 128):
                      nc.tensor.matmul(
                          yp[:], lhsT=wdt[:, kc, :],
                          rhs=prods[kc // 2][:, 256 * (kc % 2):256 * (kc % 2) + 256],
                          start=(kc == 0), stop=(kc == F // 128 - 1))
                  y_sb = workA.tile([128, T], BF, tag="y_sb", name="y_sb")
                  nc.vector.tensor_tensor(y_sb[:], yp[:], rv2_bc[:], MUL)
                  nc.vector.tensor_tensor(x[m][:], x[m][:], y_sb[:], ADD)
                  nc.scalar.activation(xb[m][:], x[m][:], Copy)

        # ---- final norm + output ----
        rbc = ln_rinv_bc(xb, D)
        for m in range(DCH):
            ot = workB.tile([128, T], FP, tag="otile", name="otile", bufs=2)
            nc.vector.tensor_tensor(ot[:], x[m][:], rbc[:], MUL)
            nc.vector.tensor_scalar(out=ot[:], in0=ot[:],
                                    scalar1=fscale[:, m:m + 1], scalar2=None, op0=MUL)
            nc.sync.dma_start(out=out_dram.ap()[128 * m:128 * (m + 1), :], in_=ot[:])

    nc.compile()
    return nc


def _get_program():
    global _PROG
    if _PROG is None:
        _PROG = _build_program()
    return _PROG


def kernel(**inputs):
    from concourse import bass_utils
    host, per_core = host_prepare(inputs)
    nc = _get_program()
    in_maps = []
    for c in range(NCORES):
        m = dict(per_core[c])
        m.update(host)
        in_maps.append(m)
    res = bass_utils.run_bass_kernel_spmd(nc, in_maps, list(range(NCORES)))
    out = np.empty((B, TQ, D), np.float32)
    for c in range(NCORES):
        grp, r = c // TP, c % TP
        out[grp, r * T:(r + 1) * T] = res.results[c]['outT'].T
    return out


# revision 24
# speedup vs baseline: 1.0689x; 1.0689x over previous
"""Self-contained Trainium2 Bass kernel for nn_EncoderDecoderTransformer_90941637525663.

Strategy: sequence-parallel over 8 NeuronCores (2 batch groups x 4 token
shards of 256 tokens). Activations live TRANSPOSED in SBUF (feature dim on
partitions, tokens on free dim); weights stream in natural [in, out] layout
as the stationary operand. All heavy matmuls run in bf16 (full PE rate +
fast weight load); stats/broadcast matmuls run f32r. Residual stream kept
fp32 in SBUF with a bf16 shadow copy for matmul use. Per-head RMS norm of
q/k makes the preceding layernorm scale cancel, so q/k projections skip LN
entirely and ln2 is never computed. One bf16 AllGather per layer exchanges
self-attention K/V shards within each batch group, overlapped with the SA
q projection and CA k/v projections. Causal masking: keep-mask zeroes V
chunks for fully-masked history, own diagonal 256x256 block handled by a
second score pass with a post-exp binary triangular mask.
"""
import sys
sys.path.insert(0, '/opt/trn_rl_repo')
import numpy as np
import ml_dtypes

BF16 = ml_dtypes.bfloat16

B, TQ, TK, D, H, KVH, L, F = 2, 1024, 512, 1024, 16, 4, 2, 4096
HD, KVD = 64, 256
EPS = 1e-6
NCORES, TP = 8, 4
T = TQ // TP           # 256 tokens per core
DCH = D // 128         # 8 feature chunks


def _rope_tables(Tlen, hd, theta=10000.0):
    freqs = 1.0 / theta ** (np.arange(0, hd, 2, dtype=np.float32) / hd)
    ang = np.outer(np.arange(Tlen, dtype=np.float32), freqs)
    return np.cos(ang).astype(np.float32), np.sin(ang).astype(np.float32)


def host_prepare(inputs):
    """Returns (host, per_core): folded shared arrays + per-core arrays."""
    inputs = {k: np.ascontiguousarray(np.asarray(v, dtype=np.float32))
              for k, v in inputs.items()}
    cos_f, sin_f = _rope_tables(TQ, HD)       # [TQ, 32]

    def wlay(w, blk):
        """[Din, N] -> [128, N//blk, Din//128, blk]: per-(partition, block)
        contiguous lines so weight DMAs use big descriptors."""
        Din, N = w.shape
        K, nb = Din // 128, N // blk
        return np.ascontiguousarray(
            w.reshape(K, 128, nb, blk).transpose(1, 2, 0, 3)).astype(BF16)

    # head-dim permutation: pair (i, i+32) -> (2i, 2i+1) so the rope swap is
    # an adjacent-pair exchange (quadrant-local => DVE stream_shuffle).
    # Dot products q.k are invariant since q and k use the same permutation.
    hsrc = np.empty(HD, np.int64)
    hsrc[0::2] = np.arange(32)
    hsrc[1::2] = np.arange(32) + 32
    qk_perm = np.concatenate([h * HD + hsrc for h in range(H)])       # for wq
    qk_perm_kv = np.concatenate([h * HD + hsrc for h in range(KVH)])  # for wk

    host = {}
    for i in range(L):
        ln1 = (1.0 + inputs['ln1_s'][i])[:, None]
        ln3 = (1.0 + inputs['ln3_s'][i])[:, None]
        # q/k rms-normalize per head, so any per-token LN scale would cancel;
        # the (identity here) ln column scales still fold into the weights.
        host[f'sa_wq_{i}'] = wlay((ln1 * inputs['sa_wq'][i])[:, qk_perm], 512)
        host[f'sa_wk_{i}'] = wlay((ln1 * inputs['sa_wk'][i])[:, qk_perm_kv], KVD)
        host[f'sa_wv_{i}'] = wlay(ln1 * inputs['sa_wv'][i], KVD)
        host[f'sa_wo_{i}'] = wlay(inputs['sa_wo'][i], 512)
        host[f'ca_wq_{i}'] = wlay(inputs['ca_wq'][i], 512)
        host[f'ca_wk_{i}'] = wlay(inputs['ca_wk'][i], KVD)
        host[f'ca_wv_{i}'] = wlay(inputs['ca_wv'][i], KVD)
        host[f'ca_wo_{i}'] = wlay(inputs['ca_wo'][i], 512)
        host[f'ffn_wg_{i}'] = wlay(ln3 * inputs['ffn_wg'][i], 512)
        host[f'ffn_wu_{i}'] = wlay(ln3 * inputs['ffn_wu'][i], 512)
        # down-proj: [m][p][kc*128+n] contiguous per output-chunk slice
        wd = inputs['ffn_wd'][i]
        host[f'ffn_wd_{i}'] = np.ascontiguousarray(
            wd.reshape(F // 128, 128, DCH, 128).transpose(2, 1, 0, 3)
            .reshape(DCH, 128, F)).astype(BF16)
        for which, dvec in [('q', inputs['sa_qn'][i]), ('k', inputs['sa_kn'][i])]:
            d1, d2 = 1.0 + dvec[:32], 1.0 + dvec[32:]
            # permuted layout: row 2i pairs with row 2i+1
            C = np.empty((HD, TQ), np.float32)
            S = np.empty((HD, TQ), np.float32)
            C[0::2] = d1[:, None] * cos_f.T
            C[1::2] = d2[:, None] * cos_f.T
            S[0::2] = -d2[:, None] * sin_f.T
            S[1::2] = d1[:, None] * sin_f.T
            # duplicated for head-pair tiles: [128, TQ]
            host[f'rope{which}_c_{i}'] = np.concatenate([C, C], 0)
            host[f'rope{which}_s_{i}'] = np.concatenate([S, S], 0)
        sc = ((1.0 + inputs['ca_qn'][i]) * (1.0 + inputs['ca_kn'][i])).astype(np.float32)
        host[f'ca_kscale_{i}'] = np.tile(sc, KVH)[:, None].copy()   # [256, 1]
    host['final_scale'] = (1.0 + inputs['final_s'])[:, None].copy()  # [D, 1]
    s2 = np.zeros((2, 128), np.float32)
    s2[0, 0:64] = 1.0
    s2[1, 64:128] = 1.0
    host['sel2const'] = s2

    # binary keep-mask for the own 256x256 causal block: [128, 2, 256] bf16
    kl = np.arange(T)[:, None]
    ql = np.arange(T)[None, :]
    tri = (kl <= ql).astype(np.float32)
    host['trimask'] = np.ascontiguousarray(
        tri.reshape(2, 128, T).transpose(1, 0, 2)).astype(BF16)

    per_core = []
    for c in range(NCORES):
        grp, r = c // TP, c % TP
        tok = slice(r * T, (r + 1) * T)
        pc = {
            'xT': np.ascontiguousarray(inputs['x'][grp].T[:, tok]),
            'xTb': np.ascontiguousarray(inputs['x'][grp].T[:, tok]).astype(BF16),
            'encT': np.ascontiguousarray(inputs['encoder_out'][grp].T).astype(BF16),
        }
        # keep-mask for pass-1 kv chunks: chunk (b, cb) kept iff 2b+cb < 2r
        vm = np.zeros((128, TP, 2), np.float32)
        for b in range(TP):
            for cb in range(2):
                vm[:, b, cb] = 1.0 if (2 * b + cb) < 2 * r else 0.0
        pc['vmask'] = vm.astype(BF16)
        for i in range(L):
            # one [128, 4, T] table per layer: (qc, qs, kc, ks)
            pc[f'rope_{i}'] = np.ascontiguousarray(np.stack(
                [host[f'ropeq_c_{i}'][:, tok], host[f'ropeq_s_{i}'][:, tok],
                 host[f'ropek_c_{i}'][:, tok], host[f'ropek_s_{i}'][:, tok]],
                axis=1)).astype(BF16)
        per_core.append(pc)
    for i in range(L):
        for which in ('q', 'k'):
            del host[f'rope{which}_c_{i}'], host[f'rope{which}_s_{i}']
    return host, per_core


_PROG = None
REPS = 1
NO_CC = False


def _build_program(nlayers=L, reps=None):
    import concourse.bass as bass
    import concourse.tile as tile
    from concourse import bacc, mybir
    from concourse.alu_op_type import AluOpType
    from contextlib import ExitStack

    R = mybir.dt.float32r
    FP = mybir.dt.float32
    BF = mybir.dt.bfloat16
    PF32 = mybir.dt.float32
    Exp = mybir.ActivationFunctionType.Exp
    Sqrt = mybir.ActivationFunctionType.Sqrt
    Square = mybir.ActivationFunctionType.Square
    Relu = mybir.ActivationFunctionType.Relu
    Copy = mybir.ActivationFunctionType.Copy

    nc = bacc.Bacc("TRN2", target_bir_lowering=False, debug=False,
                   num_devices=NCORES)

    din = {}
    def dri(name, shape, dt):
        din[name] = nc.dram_tensor(name, list(shape), dt, kind="ExternalInput")

    dri('xT', (D, T), FP)
    dri('xTb', (D, T), BF)
    dri('encT', (D, TK), BF)
    dri('trimask', (128, 2, T), BF)
    dri('vmask', (128, TP, 2), BF)
    dri('final_scale', (D, 1), FP)
    dri('sel2const', (2, 128), R)
    for i in range(nlayers):
        dri(f'sa_wq_{i}', (128, 2, DCH, 512), BF)
        dri(f'sa_wk_{i}', (128, 1, DCH, KVD), BF)
        dri(f'sa_wv_{i}', (128, 1, DCH, KVD), BF)
        dri(f'sa_wo_{i}', (128, 2, DCH, 512), BF)
        dri(f'ca_wq_{i}', (128, 2, DCH, 512), BF)
        dri(f'ca_wk_{i}', (128, 1, DCH, KVD), BF)
        dri(f'ca_wv_{i}', (128, 1, DCH, KVD), BF)
        dri(f'ca_wo_{i}', (128, 2, DCH, 512), BF)
        dri(f'ffn_wg_{i}', (128, F // 512, DCH, 512), BF)
        dri(f'ffn_wu_{i}', (128, F // 512, DCH, 512), BF)
        dri(f'ffn_wd_{i}', (DCH, 128, F), BF)
        dri(f'rope_{i}', (128, 4, T), BF)
        dri(f'ca_kscale_{i}', (KVD, 1), FP)
    out_dram = nc.dram_tensor('outT', [D, T], FP, kind="ExternalOutput")
    AGR = 2 * T + 2
    ag_in = [nc.dram_tensor(f'ag_in_{i}', [AGR, T], BF) for i in range(nlayers)]
    ag_out = [nc.dram_tensor(f'ag_out_{i}', [AGR * TP, T], BF) for i in range(nlayers)]
    own_stats = [nc.dram_tensor(f'own_stats_{i}', [T], FP) for i in range(nlayers)]
    GROUPS = [[0, 1, 2, 3], [4, 5, 6, 7]]

    with nc.allow_low_precision(reason="bf16 pipeline"), \
            tile.TileContext(nc) as tc, ExitStack() as ctx:
        consts = ctx.enter_context(tc.tile_pool(name="consts", bufs=1))
        state = ctx.enter_context(tc.tile_pool(name="state", bufs=1))
        kvf = ctx.enter_context(tc.tile_pool(name="kvf", bufs=1))
        wbig = ctx.enter_context(tc.tile_pool(name="wbig", bufs=5))
        wdp = ctx.enter_context(tc.tile_pool(name="wdp", bufs=3))
        wkv = ctx.enter_context(tc.tile_pool(name="wkv", bufs=3))
        workA = ctx.enter_context(tc.tile_pool(name="workA", bufs=2))
        workB = ctx.enter_context(tc.tile_pool(name="workB", bufs=1))
        psb = ctx.enter_context(tc.tile_pool(name="psb", bufs=5))
        ffnp = ctx.enter_context(tc.tile_pool(name="ffnp", bufs=2))
        ownp = ctx.enter_context(tc.tile_pool(name="ownp", bufs=1))
        prodp = ctx.enter_context(tc.tile_pool(name="prodp", bufs=1))
        ropep = ctx.enter_context(tc.tile_pool(name="ropep", bufs=2))
        ps = ctx.enter_context(tc.tile_pool(name="ps", bufs=8, space="PSUM"))

        def pst(p_, f_, name):
            return ps.tile([p_, f_], PF32, tag="psA", name=name)

        MUL, ADD = AluOpType.mult, AluOpType.add

        # ---- constants ----
        ones_r = consts.tile([128, 128], R, tag="ones_r", name="ones_r")
        nc.vector.memset(ones_r[:].bitcast(FP), 1.0)
        ones_b = consts.tile([128, 1], BF, tag="ones_b", name="ones_b")
        nc.vector.memset(ones_b[:], 1.0)
        bd_b = consts.tile([128, 2], BF, tag="bd_b", name="bd_b")
        nc.vector.memset(bd_b[:], 0.0)
        nc.vector.memset(bd_b[0:64, 0:1], 1.0)
        nc.vector.memset(bd_b[64:128, 1:2], 1.0)
        sel2 = consts.tile([2, 128], R, tag="sel2", name="sel2")
        nc.sync.dma_start(out=sel2[:], in_=din['sel2const'].ap())
        eps_t = consts.tile([128, 1], FP, tag="eps", name="eps")
        nc.vector.memset(eps_t[:], EPS)
        trimask = consts.tile([128, 2 * T], BF, tag="trimask", name="trimask")
        nc.sync.dma_start(out=trimask[:],
                          in_=din['trimask'].ap().rearrange("p a b -> p (a b)"))
        vmask = consts.tile([128, TP, 2], BF, tag="vmask", name="vmask")
        nc.sync.dma_start(out=vmask[:], in_=din['vmask'].ap())
        fscale = consts.tile([128, DCH], FP, tag="fscale", name="fscale")
        nc.sync.dma_start(out=fscale[:],
                          in_=din['final_scale'].ap().rearrange("(k p) o -> p (k o)", p=128))

        # ---- persistent state ----
        x = [state.tile([128, T], FP, tag=f"x{m}", name=f"x{m}") for m in range(DCH)]
        xb = [state.tile([128, T], BF, tag=f"xb{m}", name=f"xb{m}") for m in range(DCH)]
        for m in range(DCH):
            nc.sync.dma_start(out=xb[m][:], in_=din['xTb'].ap()[128 * m:128 * (m + 1), :])
        enc = [state.tile([128, TK], BF, tag=f"enc{m}", name=f"enc{m}") for m in range(DCH)]
        first_load = [True]
        ao = [state.tile([128, T], BF, tag=f"ao{m}", name=f"ao{m}") for m in range(DCH)]
        qf = [state.tile([128, T], BF, tag=f"qf{t}", name=f"qf{t}") for t in range(H // 2)]
        kdup = [state.tile([128, T], BF, tag=f"kd{k}", name=f"kd{k}") for k in range(KVH)]
        v_own = [state.tile([128, 2, 65], BF, tag=f"vo{k}", name=f"vo{k}") for k in range(KVH)]
        kdca = [state.tile([128, TK], BF, tag=f"kdca{k}", name=f"kdca{k}") for k in range(KVH)]
        cv = [state.tile([128, TP, 65], BF, tag=f"cv{k}", name=f"cv{k}") for k in range(KVH)]

        # ---------------- helpers ----------------
        def ln_ss(src_tiles):
            """Sum of squares over D partitions -> [1,T] psum."""
            ss = pst(1, T, "ss")
            for m in range(DCH):
                sq = workA.tile([128, T], BF, tag="sq", name="sq")
                nc.scalar.activation(sq[:], src_tiles[m][:], Square)
                nc.tensor.matmul(ss[:], lhsT=ones_b[:, 0:1], rhs=sq[:],
                                 start=(m == 0), stop=(m == DCH - 1))
            return ss

        def ln_rinv(src_tiles, n):
            """1/rms over D partitions -> rinv [1,T] fp32."""
            ss = ln_ss(src_tiles)
            sr = workB.tile([1, T], FP, tag="sr", name="sr")
            nc.scalar.activation(sr[:], ss[:], Sqrt, bias=eps_t[0:1, :], scale=1.0 / n)
            rinv = workB.tile([1, T], FP, tag="rinv", name="rinv")
            nc.vector.reciprocal_approx_fast(out=rinv[:], in_=sr[:])
            return rinv

        def ln_rinv_bc(src_tiles, n):
            """1/rms over D partitions, broadcast -> [128,T] fp32 sbuf."""
            ss = ln_ss(src_tiles)
            srr = workB.tile([1, T], R, tag="srr", name="srr")
            nc.scalar.activation(srr[:], ss[:], Sqrt, bias=eps_t[0:1, :], scale=1.0 / n)
            bc_ps = pst(128, T, "bc")
            nc.tensor.matmul(bc_ps[:], lhsT=ones_r[0:1, :], rhs=srr[:],
                             start=True, stop=True)
            rbc = workA.tile([128, T], FP, tag="rbcf", name="rbcf")
            nc.vector.reciprocal_approx_fast(out=rbc[:], in_=bc_ps[:])
            return rbc

        def head_pair_rms(q_ps, width):
            """Per-head 1/rms of a [128, width] psum (2 heads) -> fp32 bcast
            [128, width] sbuf tile."""
            sq = workA.tile([128, width], BF, tag="sqh", name="sqh", padded_shape=[128, TK])
            nc.scalar.activation(sq[:], q_ps[:], Square)
            ssq = pst(2, width, "ssq")
            nc.tensor.matmul(ssq[:], lhsT=bd_b[:], rhs=sq[:], start=True, stop=True)
            sr = workB.tile([2, width], R, tag="srh", name="srh", padded_shape=[2, TK])
            nc.scalar.activation(sr[:], ssq[:], Sqrt, bias=eps_t[0:2, :], scale=1.0 / HD)
            bc_ps = pst(128, width, "bch")
            nc.tensor.matmul(bc_ps[:], lhsT=sel2[:], rhs=sr[:], start=True, stop=True)
            rbc = workA.tile([128, width], FP, tag="rbch", name="rbch", padded_shape=[128, TK])
            nc.vector.reciprocal_approx_fast(out=rbc[:], in_=bc_ps[:])
            return rbc

        SWAP_MASK = [j ^ 1 for j in range(32)]

        def apply_rope(dst, qhat, c_t, s_t):
            qsw = workA.tile([128, T], BF, tag="qsw", name="qsw")
            nc.vector.stream_shuffle(qsw[:], qhat[:], SWAP_MASK)
            nc.vector.tensor_tensor(qhat[:], qhat[:], c_t, MUL)
            nc.vector.tensor_tensor(qsw[:], qsw[:], s_t, MUL)
            nc.vector.tensor_tensor(dst, qhat[:], qsw[:], ADD)

        def proj(dst_eval, w_name, ncols, blk=512):
            nblk = ncols // blk
            for bki in range(nblk):
                pool_, tg = (wbig, "wbig") if blk == 512 else (wkv, "wkv")
                wt = pool_.tile([128, DCH, blk], BF, tag=tg, name=tg)
                nc.sync.dma_start(out=wt[:], in_=din[w_name].ap()[:, bki])
                for j in range(blk // 128):
                    mt = bki * (blk // 128) + j
                    q_ps = pst(128, T, "proj")
                    for k in range(DCH):
                        nc.tensor.matmul(q_ps[:], lhsT=wt[:, k, 128 * j:128 * (j + 1)],
                                         rhs=xb[k][:],
                                         start=(k == 0), stop=(k == DCH - 1))
                    dst_eval(mt, q_ps)

        def own_precompute(i):
            """Exp'd+masked own-diagonal probability tiles for all 16 heads.
            Emitted before any AllGather-dependent matmul so the PE queue has
            work during the collective."""
            p2_own = {}
            for kv in range(KVH):
                for pr in range(2):
                    qt = kv * 2 + pr
                    s_pair = [pst(128, 512, "s"), pst(128, 512, "s")]
                    for cc in range(2):
                        for hp in range(2):
                            par = 64 * hp
                            nc.tensor.matmul(
                                s_pair[hp][:, 256 * cc:256 * (cc + 1)],
                                lhsT=kdup[kv][par:par + 64,
                                              128 * cc:128 * (cc + 1)],
                                rhs=qf[qt][par:par + 64, :],
                                start=True, stop=True)
                    for hp in range(2):
                        pe = psb.tile([128, 512], BF, tag="pe", name="pe", bufs=3)
                        nc.scalar.activation(pe[:], s_pair[hp][:], Exp, scale=0.125)
                        p2 = ownp.tile([128, 512], BF, tag=f"p2o{qt}_{hp}",
                                       name=f"p2o{qt}_{hp}")
                        nc.vector.tensor_tensor(p2[:], pe[:], trimask[:], MUL)
                        p2_own[2 * qt + hp] = p2
            attention.p2_own = p2_own

        def attention(i, is_sa):
            pending = []

            def flush_norm():
                for qt, hp, o_ps in pending:
                    par = 64 * hp
                    r_sb = workB.tile([65, T], R, tag="r_sb", name="r_sb", bufs=3)
                    nc.scalar.activation(r_sb[64:65, :], o_ps[64:65, :], Copy)
                    b_ps = pst(64, T, "b")
                    nc.tensor.matmul(b_ps[:], lhsT=ones_r[64:65, 0:64],
                                     rhs=r_sb[64:65, :], start=True, stop=True)
                    b_sb = workB.tile([64, T], FP, tag="b_sb", name="b_sb", bufs=3)
                    nc.vector.reciprocal_approx_fast(out=b_sb[:], in_=b_ps[:])
                    o_scr = workB.tile([64, T], BF, tag="o_scr", name="o_scr",
                                       bufs=2)
                    nc.vector.tensor_tensor(o_scr[:], o_ps[0:64, :], b_sb[:], MUL)
                    nc.gpsimd.dma_start(out=ao[qt][par:par + 64, :], in_=o_scr[:])
                pending.clear()

            for kv in range(KVH):
                if is_sa:
                    ag = ag_out[i].ap()
                    ksrc = kvf.tile([128, TQ], BF, tag="kfull", name="kfull")
                    src = bass.AP(tensor=ag.tensor, offset=(64 * kv) * T,
                                  ap=[[T, 64], [AGR * T, TP], [1, T]])
                    for dd in range(2):
                        nc.sync.dma_start(
                            out=ksrc[64 * dd:64 * (dd + 1), :].rearrange(
                                "p (b t) -> p b t", b=TP), in_=src)
                    vsrc = kvf.tile([128, TP, 2, 65], BF, tag="vfull", name="vfull")
                    for cb in range(2):
                        vap = bass.AP(tensor=ag.tensor,
                                      offset=T * T + 128 * T * cb + 64 * kv,
                                      ap=[[T, 128], [AGR * T, TP], [1, 64]])
                        nc.sync.dma_start(out=vsrc[:, :, cb, 0:64], in_=vap)
                    if kv == 0:
                        rvg = workB.tile([128, TP, 2], BF, tag="rvg", name="rvg")
                        for cb in range(2):
                            rap = bass.AP(tensor=ag.tensor, offset=2 * T * T + 128 * cb,
                                          ap=[[1, 128], [AGR * T, TP]])
                            nc.sync.dma_start(out=rvg[:, :, cb], in_=rap)
                        nc.vector.tensor_tensor(rvg[:], rvg[:], vmask[:], MUL)
                        attention.rvg = rvg
                    rvg = attention.rvg
                    nc.vector.tensor_tensor(
                        vsrc[:, :, :, 0:64], vsrc[:, :, :, 0:64],
                        rvg[:, :, :, None].broadcast_to([128, TP, 2, 64]), MUL)
                    nc.vector.tensor_tensor(
                        vsrc[:, :, :, 64:65], vmask[:, :, :, None],
                        vmask[:, :, :, None], MUL)
                    nk_chunks = TQ // 128
                else:
                    ksrc = kdca[kv]
                    vsrc = cv[kv]
                    nk_chunks = TK // 128
                # two heads (par 0 / par 64) pipelined: score matmuls of a
                # stage issue back-to-back on disjoint PE row groups (runs
                # concurrently), o-accumulation trails one stage behind so
                # the softmax exp is off the PE critical path. The own
                # diagonal block (precomputed) is the last accumulation stage.
                for pr in range(2):
                    qt = kv * 2 + pr
                    o_pair = [None, None]
                    nmm = nk_chunks + (2 if is_sa else 0)
                    stages = [('hist', h) for h in range(nk_chunks // 2)] + \
                        (['own'] if is_sa else [])
                    nst = len(stages)
                    mi = [0, 0]
                    prev = None
                    for si in range(nst + 1):
                        cur = None
                        if si < nst:
                            st = stages[si]
                            if st == 'own':
                                cur = (st, [attention.p2_own[2 * qt],
                                            attention.p2_own[2 * qt + 1]])
                            else:
                                s_pair = [pst(128, 512, "s"), pst(128, 512, "s")]
                                for cc in range(2):
                                    for hp in range(2):
                                        par = 64 * hp
                                        c = 2 * st[1] + cc
                                        nc.tensor.matmul(
                                            s_pair[hp][:, 256 * cc:256 * (cc + 1)],
                                            lhsT=ksrc[par:par + 64,
                                                      128 * c:128 * (c + 1)],
                                            rhs=qf[qt][par:par + 64, :],
                                            start=True, stop=True)
                                p_pair = []
                                for hp in range(2):
                                    p_sb = psb.tile([128, 512], BF, tag="p_sb",
                                                    name="p_sb")
                                    nc.scalar.activation(p_sb[:], s_pair[hp][:],
                                                         Exp, scale=0.125)
                                    p_pair.append(p_sb)
                                cur = (st, p_pair)
                        if si == 0:
                            flush_norm()   # previous pair's normalize queues
                                           # behind this pair's first scores
                        if prev is not None:
                            pst_, pp = prev
                            if o_pair[0] is None:
                                o_pair[0] = pst(65, T, "o")
                                o_pair[1] = pst(65, T, "o")
                            for hp in range(2):
                                for cc in range(2):
                                    if pst_ == 'own':
                                        vsl = v_own[kv][:, cc, :]
                                    else:
                                        c = 2 * pst_[1] + cc
                                        vsl = (vsrc[:, c // 2, c % 2, :]
                                               if is_sa else vsrc[:, c, :])
                                    nc.tensor.matmul(
                                        o_pair[hp][:],
                                        lhsT=vsl,
                                        rhs=pp[hp][:, 256 * cc:256 * (cc + 1)],
                                        start=(mi[hp] == 0),
                                        stop=(mi[hp] == nmm - 1),
                                        skip_group_check=True)
                                    mi[hp] += 1
                        prev = cur
                    for hp in range(2):
                        pending.append((qt, hp, o_pair[hp]))
            flush_norm()

        def stream_out_proj(w_name):
            for bki in range(2):
                wt = wbig.tile([128, DCH, 512], BF, tag="wbig", name="wbig")
                nc.sync.dma_start(out=wt[:], in_=din[w_name].ap()[:, bki])
                for j in range(4):
                    m = bki * 4 + j
                    y_ps = pst(128, T, "proj")
                    for k in range(DCH):
                        nc.tensor.matmul(y_ps[:], lhsT=wt[:, k, 128 * j:128 * (j + 1)],
                                         rhs=ao[k][:],
                                         start=(k == 0), stop=(k == DCH - 1))
                    nc.vector.tensor_tensor(x[m][:], x[m][:], y_ps[:], ADD)
                    nc.scalar.activation(xb[m][:], x[m][:], Copy)

        # ================= layers (REPS > 1 only for timing runs) =================
        for rep in range(reps if reps is not None else REPS):
          if rep > 0:
            for m in range(DCH):
                nc.sync.dma_start(out=x[m][:], in_=din['xT'].ap()[128 * m:128 * (m + 1), :])
                nc.sync.dma_start(out=xb[m][:], in_=din['xTb'].ap()[128 * m:128 * (m + 1), :])
          for i in range(nlayers):
              # ---- LN1 stats (only needed for v scaling) ----
              rinv = ln_rinv(xb, D)
              nc.sync.dma_start(out=own_stats[i].ap()[None, :], in_=rinv[:])
              rin_b = workB.tile([1, T], BF, tag="rin_b", name="rin_b")
              nc.vector.tensor_copy(out=rin_b[:], in_=rinv[:])
              nc.sync.dma_start(out=ag_in[i].ap()[2 * T:2 * T + 1, :], in_=rin_b[:])

              ropet = ropep.tile([128, 4, T], BF, tag="ropet", name="ropet")
              nc.sync.dma_start(out=ropet[:], in_=din[f'rope_{i}'].ap())

              # ---- SA K projection (LN cancels under per-head rms) ----
              kf_pair = [workB.tile([128, T], BF, tag=f"kfp{t}", name=f"kfp{t}")
                         for t in range(2)]
              def k_eval(t, k_ps):
                  rbch = head_pair_rms(k_ps, T)
                  khat = workA.tile([128, T], BF, tag="qhat", name="qhat")
                  nc.vector.tensor_tensor(khat[:], k_ps[:], rbch[:], MUL)
                  apply_rope(kf_pair[t][:], khat, ropet[:, 2, :], ropet[:, 3, :])
              proj(k_eval, f'sa_wk_{i}', KVD, blk=KVD)
              for t in range(2):
                  nc.sync.dma_start(out=ag_in[i].ap()[128 * t:128 * (t + 1), :],
                                    in_=kf_pair[t][:])
                  for half in range(2):
                      kv = 2 * t + half
                      for dd in range(2):
                          nc.gpsimd.dma_start(out=kdup[kv][64 * dd:64 * (dd + 1), :],
                                              in_=kf_pair[t][64 * half:64 * (half + 1), :])

              # ---- SA V projection ----
              wvt = wkv.tile([128, DCH, KVD], BF, tag="wkv", name="wkv")
              nc.sync.dma_start(out=wvt[:], in_=din[f'sa_wv_{i}'].ap()[:, 0])
              for j in range(2):
                  v_ps = pst(128, KVD, "proj")
                  for k in range(DCH):
                      nc.tensor.matmul(v_ps[:], lhsT=xb[k][:, 128 * j:128 * (j + 1)],
                                       rhs=wvt[:, k, :], start=(k == 0), stop=(k == DCH - 1))
                  for kv in range(KVH):
                      nc.scalar.activation(v_own[kv][:, j, 0:64],
                                           v_ps[:, 64 * kv:64 * (kv + 1)], Copy)
              # own-block v scaling by rinv (token-on-partition, via dram bounce)
              for j in range(2):
                  rvT = workB.tile([128, 1], FP, tag="rvT", name="rvT", bufs=2)
                  nc.gpsimd.dma_start(out=rvT[:],
                                    in_=own_stats[i].ap()[128 * j:128 * (j + 1), None])
                  for kv in range(KVH):
                      nc.vector.tensor_scalar(
                          out=v_own[kv][:, j, 0:64],
                          in0=v_own[kv][:, j, 0:64],
                          scalar1=rvT[:], scalar2=None, op0=MUL)
              for kv in range(KVH):
                  nc.vector.memset(v_own[kv][:, :, 64:65], 1.0)
              for kv in range(KVH):
                  for j in range(2):
                      nc.gpsimd.dma_start(
                          out=ag_in[i].ap()[T + 128 * j:T + 128 * (j + 1),
                                            64 * kv:64 * (kv + 1)],
                          in_=v_own[kv][:, j, 0:64])

              if NO_CC:
                  for b in range(TP):
                      nc.sync.dma_start(
                          out=ag_out[i].ap()[AGR * b:AGR * (b + 1), :],
                          in_=ag_in[i].ap())
              else:
                  nc.gpsimd.collective_compute(
                      "AllGather", mybir.AluOpType.bypass, replica_groups=GROUPS,
                      ins=[ag_in[i].ap().opt()], outs=[ag_out[i].ap().opt()])
              if first_load[0]:
                  first_load[0] = False
                  for m in range(DCH):
                      nc.sync.dma_start(out=x[m][:],
                                        in_=din['xT'].ap()[128 * m:128 * (m + 1), :])
                      nc.sync.dma_start(out=enc[m][:],
                                        in_=din['encT'].ap()[128 * m:128 * (m + 1), :])

              # ---- overlap the AllGather: SA Q proj + CA K/V (enc-only) ----
              def q_eval(t, q_ps):
                  rbch = head_pair_rms(q_ps, T)
                  qhat = workA.tile([128, T], BF, tag="qhat", name="qhat")
                  nc.vector.tensor_tensor(qhat[:], q_ps[:], rbch[:], MUL)
                  apply_rope(qf[t][:], qhat, ropet[:, 0, :], ropet[:, 1, :])
              proj(q_eval, f'sa_wq_{i}', D)
              own_precompute(i)

              ksc = workB.tile([128, 2], FP, tag="ksc", name="ksc")
              nc.sync.dma_start(out=ksc[:],
                                in_=din[f'ca_kscale_{i}'].ap().rearrange("(t p) o -> p (t o)", p=128))
              wkt = wkv.tile([128, DCH, KVD], BF, tag="wkv", name="wkv")
              nc.sync.dma_start(out=wkt[:], in_=din[f'ca_wk_{i}'].ap()[:, 0])
              for t in range(2):
                  k_ps = pst(128, TK, "s")
                  for k in range(DCH):
                      nc.tensor.matmul(k_ps[:], lhsT=wkt[:, k, 128 * t:128 * (t + 1)],
                                       rhs=enc[k][:], start=(k == 0), stop=(k == DCH - 1))
                  rbch = head_pair_rms(k_ps, TK)
                  kh = workB.tile([128, TK], BF, tag="khca", name="khca")
                  nc.vector.tensor_tensor(kh[:], k_ps[:], rbch[:], MUL)
                  ckp = workB.tile([128, TK], BF, tag=f"ckp{t}", name=f"ckp{t}")
                  nc.vector.tensor_scalar(
                      out=ckp[:], in0=kh[:],
                      scalar1=ksc[:, t:t + 1], scalar2=None, op0=MUL)
                  for half in range(2):
                      kv = 2 * t + half
                      for dd in range(2):
                          nc.gpsimd.dma_start(out=kdca[kv][64 * dd:64 * (dd + 1), :],
                                              in_=ckp[64 * half:64 * (half + 1), :])

              wvt2 = wkv.tile([128, DCH, KVD], BF, tag="wkv", name="wkv")
              nc.sync.dma_start(out=wvt2[:], in_=din[f'ca_wv_{i}'].ap()[:, 0])
              for kv in range(KVH):
                  nc.vector.memset(cv[kv][:, :, 64:65], 1.0)
              for j in range(TP):
                  v_ps = pst(128, KVD, "proj")
                  for k in range(DCH):
                      nc.tensor.matmul(v_ps[:], lhsT=enc[k][:, 128 * j:128 * (j + 1)],
                                       rhs=wvt2[:, k, :], start=(k == 0), stop=(k == DCH - 1))
                  for kv in range(KVH):
                      nc.scalar.activation(cv[kv][:, j, 0:64],
                                           v_ps[:, 64 * kv:64 * (kv + 1)], Copy)

              # ---- SA attention + out-proj ----
              attention(i, True)
              stream_out_proj(f'sa_wo_{i}')

              # ---- CA Q (LN2 cancels entirely) + attention + out-proj ----
              def cq_eval(t, q_ps):
                  rbch = head_pair_rms(q_ps, T)
                  nc.vector.tensor_tensor(qf[t][:], q_ps[:], rbch[:], MUL)
              proj(cq_eval, f'ca_wq_{i}', D)
              attention(i, False)
              stream_out_proj(f'ca_wo_{i}')

              # ---- LN3 + FFN ----
              rbc3 = ln_rinv_bc(xb, D)
              rv2_bc = workA.tile([128, T], BF, tag="rv2bc", name="rv2bc")
              nc.vector.tensor_tensor(rv2_bc[:], rbc3[:], rbc3[:], MUL)

              prods = []
              NF = F // 512
              for fb in range(NF):
                  wgt = wbig.tile([128, DCH, 512], BF, tag="wbig", name="wbig")
                  nc.sync.dma_start(out=wgt[:], in_=din[f'ffn_wg_{i}'].ap()[:, fb])
                  wut = wbig.tile([128, DCH, 512], BF, tag="wbig", name="wbig")
                  nc.sync.dma_start(out=wut[:], in_=din[f'ffn_wu_{i}'].ap()[:, fb])
                  for hf in range(2):
                      gu = []
                      for which, wt in (('g', wgt), ('u', wut)):
                          g_ps = pst(128, 512, "s")
                          for jj in range(2):
                              j = 2 * hf + jj
                              for k in range(DCH):
                                  nc.tensor.matmul(
                                      g_ps[:, 256 * jj:256 * (jj + 1)],
                                      lhsT=wt[:, k, 128 * j:128 * (j + 1)],
                                      rhs=xb[k][:], start=(k == 0), stop=(k == DCH - 1))
                          g_sb = ffnp.tile([128, 512], BF, tag=f"relu{which}", name=f"relu{which}")
                          nc.scalar.activation(g_sb[:], g_ps[:], Relu)
                          gu.append(g_sb)
                      pr = prodp.tile([128, 512], BF, tag=f"prod{fb}_{hf}",
                                      name=f"prod{fb}_{hf}")
                      nc.vector.tensor_tensor(pr[:], gu[0][:], gu[1][:], MUL)
                      prods.append(pr)
              # down-proj: m-outer, full-K accumulation (no partial adds)
              for m in range(DCH):
                  wdt = wdp.tile([128, F // 128, 128], BF, tag="wdp", name="wdp")
                  nc.sync.dma_start(
                      out=wdt[:].rearrange("p a b -> p (a b)"),
                      in_=din[f'ffn_wd_{i}'].ap()[m])
                  yp = pst(128, T, "yp")
                  for kc in range(F // 128):
                      nc.tensor.matmul(
                          yp[:], lhsT=wdt[:, kc, :],
                          rhs=prods[kc // 2][:, 256 * (kc % 2):256 * (kc % 2) + 256],
                          start=(kc == 0), stop=(kc == F // 128 - 1))
                  y_sb = workA.tile([128, T], BF, tag="y_sb", name="y_sb")
                  nc.vector.tensor_tensor(y_sb[:], yp[:], rv2_bc[:], MUL)
                  nc.vector.tensor_tensor(x[m][:], x[m][:], y_sb[:], ADD)
                  nc.scalar.activation(xb[m][:], x[m][:], Copy)

        # ---- final norm + output ----
        rbc = ln_rinv_bc(xb, D)
        for m in range(DCH):
            ot = workB.tile([128, T], FP, tag="otile", name="otile", bufs=2)
            nc.vector.tensor_tensor(ot[:], x[m][:], rbc[:], MUL)
            nc.vector.tensor_scalar(out=ot[:], in0=ot[:],
                                    scalar1=fscale[:, m:m + 1], scalar2=None, op0=MUL)
            nc.sync.dma_start(out=out_dram.ap()[128 * m:128 * (m + 1), :], in_=ot[:])

    nc.compile()
    return nc


def _get_program():
    global _PROG
    if _PROG is None:
        _PROG = _build_program()
    return _PROG


def kernel(**inputs):
    from concourse import bass_utils
    host, per_core = host_prepare(inputs)
    nc = _get_program()
    in_maps = []
    for c in range(NCORES):
        m = dict(per_core[c])
        m.update(host)
        in_maps.append(m)
    res = bass_utils.run_bass_kernel_spmd(nc, in_maps, list(range(NCORES)))
    out = np.empty((B, TQ, D), np.float32)
    for c in range(NCORES):
        grp, r = c // TP, c % TP
        out[grp, r * T:(r + 1) * T] = res.results[c]['outT'].T
    return out
